# revision 10
# baseline (speedup 1.0000x reference)
"""Trainium2 Bass kernel for nn_CPDecode: 5-layer single-token decode + 15 LM heads.

Sharding (8 cores, tensor parallel):
  - attention: 1 head per core (H=KVH=8); Wq/Wk/Wv rows, Wo cols, kv_cache head dim
  - MLP: FF/8 = 512 rows of Wg/Wu per core, 512 cols of Wd
  - lm_heads: V/8 = 256 rows of each of the 15 heads per core
  - AllReduce (ncfw) after o_proj and down_proj partials; logits/new-cache
    assembled on host.

All weights are pre-transposed on host so every PE contraction has K on the
partition axis. 1024-vectors live in SBUF as [128, 8] "col-block" tiles:
tile[p, g] = v[g*128 + p].
"""

import sys
import os

sys.path.insert(0, "/opt/trn_rl_repo")

import numpy as np
import ml_dtypes

L, H, KVH, D, HID, FF, V, NCB, S = 5, 8, 8, 128, 1024, 4096, 2048, 15, 4096
BASE, EPS = 10000.0, 1e-6
NC = 8
G = HID // 128  # 8 col-blocks of the residual stream
FG = 512 // 128  # 4 col-blocks of the per-core FF shard
SB = S // 128  # 32 score blocks
LMR = 8  # lm ranges
LMN = 3840 // LMR  # 480 outputs per range

_COMPILED = None


def _build():
    from concourse import bass, bacc, tile, mybir

    F32 = mybir.dt.float32
    BF16 = mybir.dt.bfloat16
    AL = mybir.AluOpType
    ACTF = mybir.ActivationFunctionType

    nc = bacc.Bacc("TRN2", target_bir_lowering=False, debug=False, num_devices=NC)

    # ---- I/O ----
    x0_in = nc.dram_tensor("x0", [128, G], F32, kind="ExternalInput")
    csq_in = nc.dram_tensor("csq", [1, 256], F32, kind="ExternalInput")
    csk_in = nc.dram_tensor("csk", [1, 256], F32, kind="ExternalInput")
    lnw_in = nc.dram_tensor("lnw", [128, G, 2 * L + 1], F32, kind="ExternalInput")
    wqkv_in = nc.dram_tensor("wqkv", [128, L, G, 384], BF16, kind="ExternalInput")
    wo_in = nc.dram_tensor("wo", [128, L, G, 128], BF16, kind="ExternalInput")
    wg_in = nc.dram_tensor("wg", [128, L, G, 512], BF16, kind="ExternalInput")
    wu_in = nc.dram_tensor("wu", [128, L, G, 512], BF16, kind="ExternalInput")
    wd_in = nc.dram_tensor("wd", [128, L, FG, G, 128], BF16, kind="ExternalInput")
    kt_in = nc.dram_tensor("kt", [L, 128, S], BF16, kind="ExternalInput")
    vc_in = nc.dram_tensor("vc", [L, SB, 128, 128], BF16, kind="ExternalInput")
    lm_in = nc.dram_tensor("lmw", [128, G, LMR, LMN], BF16, kind="ExternalInput")

    lgt_out = nc.dram_tensor("lgt", [LMR, LMN], F32, kind="ExternalOutput")
    kvn_out = nc.dram_tensor("kvn", [L, 2, 128], F32, kind="ExternalOutput")

    with tile.TileContext(nc) as tc:
        with (
            tc.tile_pool(name="const", bufs=1) as cpool,
            tc.tile_pool(name="wb", bufs=2) as wpool,
            tc.tile_pool(name="act", bufs=2) as apool,
            tc.tile_pool(name="ps", bufs=1, space="PSUM") as pp,
            tc.tile_pool(name="dram", bufs=1, space="DRAM") as dpool,
        ):
            # ---- constants ----
            csq = cpool.tile([1, 256], F32)
            nc.sync.dma_start(csq[:], csq_in.ap())
            csk = cpool.tile([1, 256], F32)
            nc.sync.dma_start(csk[:], csk_in.ap())
            lnw = cpool.tile([128, G, 2 * L + 1], F32)
            nc.sync.dma_start(lnw[:], lnw_in.ap())
            ones_c = cpool.tile([128, 1], F32)
            nc.vector.memset(ones_c[:], 1.0)
            ones_r = cpool.tile([1, 128], F32)
            nc.vector.memset(ones_r[:], 1.0)
            eps_t = cpool.tile([1, 1], F32)
            nc.vector.memset(eps_t[:], EPS)

            x = apool.tile([128, G], F32, name="x_init", tag="x", bufs=3)
            nc.sync.dma_start(x[:], x0_in.ap())

            # ACT table pre-warm scratch
            junk = cpool.tile([1, 1], F32)
            nc.vector.memset(junk[:], 1.0)
            junko = cpool.tile([1, 4], F32)

            def warm(func):
                nc.scalar.activation(junko[:, 0:1], junk[:], func)

            def rms_scale(xt, tag):
                """rsqrt(mean(x^2)+eps) -> ([1,1] sbuf, [128,1] sbuf bcast)"""
                sq = apool.tile([128, G], F32, name=f"sq_{tag}", tag="sq")
                nc.vector.tensor_tensor(sq[:], xt[:], xt[:], AL.mult)
                rs = apool.tile([128, 1], F32, name=f"rs_{tag}", tag="rs")
                nc.vector.tensor_reduce(rs[:], sq[:], mybir.AxisListType.X, AL.add)
                ssq = pp.tile([1, 1], F32, name=f"ssq_{tag}", tag="scalP")
                nc.tensor.matmul(ssq[:], rs[:], ones_c[:], start=True, stop=True)
                sr = apool.tile([1, 1], F32, name=f"sr_{tag}", tag="sr", bufs=2)
                nc.scalar.activation(sr[:], ssq[:], ACTF.Sqrt, bias=eps_t[:], scale=1.0 / HID)
                s = apool.tile([1, 1], F32, name=f"s_{tag}", tag="s", bufs=3)
                nc.vector.reciprocal(s[:], sr[:])
                sbcp = pp.tile([128, 1], F32, name=f"sbcp_{tag}", tag="vecP", bufs=2)
                nc.tensor.matmul(sbcp[:], ones_r[:], s[:], start=True, stop=True)
                sbc = apool.tile([128, 1], F32, name=f"sbc_{tag}", tag="sbc", bufs=2)
                nc.vector.tensor_copy(sbc[:], sbcp[:])
                return s, sbc

            def rope_row(dst, src_ap, cs):
                """dst[1,128] = src*cos + rot_half(src)*sin (row form)."""
                qc = apool.tile([1, 128], F32, name=f"{dst.name}_qc", tag="ropeA")
                nc.vector.tensor_tensor(qc[:], src_ap, cs[:, 0:128], AL.mult)
                t = apool.tile([1, 128], F32, name=f"{dst.name}_t", tag="ropeB")
                nc.vector.tensor_tensor(t[:, 0:64], src_ap[:, 64:128], cs[:, 128:192], AL.mult)
                nc.vector.tensor_tensor(t[:, 64:128], src_ap[:, 0:64], cs[:, 192:256], AL.mult)
                nc.vector.tensor_tensor(dst[:], qc[:], t[:], AL.add)

            for i in range(L):
                # ==== attention ====
                s1, s1bc = rms_scale(x, f"l{i}a")
                ht = apool.tile([128, G], BF16, name=f"ht_{i}", tag="ht")
                nc.vector.tensor_tensor(ht[:], x[:], lnw[:, :, 2 * i], AL.mult)
                warm(ACTF.Exp)

                wqkv_t = wpool.tile([128, G, 384], BF16, name=f"wqkv_{i}", tag="wqkv")
                nc.sync.dma_start(wqkv_t[:], wqkv_in.ap()[:, i])
                qkv = pp.tile([1, 384], F32, name=f"qkv_{i}", tag="qkvrow")
                for g in range(G):
                    nc.tensor.matmul(
                        qkv[:], ht[:, g : g + 1], wqkv_t[:, g, :],
                        start=(g == 0), stop=(g == G - 1),
                    )

                # rope (row form, unscaled by s1)
                qr = apool.tile([1, 128], F32, name=f"qr_{i}", tag="qr")
                rope_row(qr, qkv[:, 0:128], csq)
                kr = apool.tile([1, 128], F32, name=f"kr_{i}", tag="kr")
                rope_row(kr, qkv[:, 128:256], csk)

                # kv outputs: true k/v = s1 * (unscaled)
                kn = apool.tile([1, 128], F32, name=f"kn_{i}", tag="kn")
                nc.vector.tensor_scalar(kn[:], kr[:], s1[:], None, AL.mult)
                vn = apool.tile([1, 128], F32, name=f"vn_{i}", tag="vn")
                nc.vector.tensor_scalar(vn[:], qkv[:, 256:384], s1[:], None, AL.mult)
                nc.sync.dma_start(kvn_out.ap()[i, 0], kn[:])
                nc.sync.dma_start(kvn_out.ap()[i, 1], vn[:])

                # transpose qr, kr to columns
                qrcp = pp.tile([128, 1], F32, name=f"qrcp_{i}", tag="vecP", bufs=2)
                nc.tensor.matmul(qrcp[:], qr[:], ones_c[0:1, :], start=True, stop=True)
                qrc = apool.tile([128, 1], BF16, name=f"qrc_{i}", tag="qrc")
                nc.vector.tensor_copy(qrc[:], qrcp[:])
                krcp = pp.tile([128, 1], F32, name=f"krcp_{i}", tag="vecP", bufs=2)
                nc.tensor.matmul(krcp[:], kr[:], ones_c[0:1, :], start=True, stop=True)
                krc = apool.tile([128, 1], BF16, name=f"krc_{i}", tag="krc")
                nc.vector.tensor_copy(krc[:], krcp[:])

                # scores: [128, 32] blocks vs cached K^T
                ktile = wpool.tile([128, S], BF16, name=f"kt_{i}", tag="kt", bufs=1)
                nc.sync.dma_start(ktile[:], kt_in.ap()[i])
                sacc = pp.tile([128, SB], F32, name=f"sacc_{i}", tag="sacc")
                for b in range(SB):
                    nc.tensor.matmul(
                        sacc[:, b : b + 1], ktile[:, b * 128 : (b + 1) * 128],
                        qrc[:], start=True, stop=True,
                    )
                snew = pp.tile([1, 1], F32, name=f"snew_{i}", tag="scalP")
                nc.tensor.matmul(snew[:], qrc[:], krc[:], start=True, stop=True)

                # softmax (unnormalized; scale folds s1 back in)
                es = apool.tile([128, SB], BF16, name=f"es_{i}", tag="es")
                rowsum = apool.tile([128, 1], F32, name=f"rsum_{i}", tag="rsum")
                nc.scalar.activation(
                    es[:], sacc[:], ACTF.Exp, scale=s1bc[:], accum_out=rowsum[:]
                )
                s1sq = apool.tile([1, 1], F32, name=f"s1sq_{i}", tag="s1sq")
                nc.vector.tensor_tensor(s1sq[:], s1[:], s1[:], AL.mult)
                enew = apool.tile([1, 1], F32, name=f"enew_{i}", tag="enew")
                nc.scalar.activation(enew[:], snew[:], ACTF.Exp, scale=s1sq[:])
                warm(ACTF.Sqrt)
                dsum = pp.tile([1, 1], F32, name=f"dsum_{i}", tag="scalP")
                nc.tensor.matmul(dsum[:], rowsum[:], ones_c[:], start=True, stop=True)
                dtot = apool.tile([1, 1], F32, name=f"dtot_{i}", tag="dtot")
                nc.vector.tensor_tensor(dtot[:], dsum[:], enew[:], AL.add)
                rr = apool.tile([1, 1], F32, name=f"rr_{i}", tag="rr")
                nc.vector.reciprocal(rr[:], dtot[:])

                # attn @ V -> o row [1, 128]
                vtile = wpool.tile([128, SB, 128], BF16, name=f"v_{i}", tag="vt", bufs=1)
                nc.sync.dma_start(vtile[:], vc_in.ap()[i].rearrange("b s d -> s b d"))
                orow = pp.tile([1, 128], F32, name=f"orow_{i}", tag="qkvrow")
                for b in range(SB):
                    nc.tensor.matmul(
                        orow[:], es[:, b : b + 1], vtile[:, b, :],
                        start=(b == 0), stop=(b == SB - 1),
                    )
                # o = (orow + enew * vn) * rr
                osb = apool.tile([1, 128], F32, name=f"osb_{i}", tag="osb")
                nc.vector.scalar_tensor_tensor(
                    osb[:], vn[:], enew[:], orow[:], AL.mult, AL.add
                )
                nc.vector.tensor_scalar(osb[:], osb[:], rr[:], None, AL.mult)
                # transpose to column
                ocp = pp.tile([128, 1], F32, name=f"ocp_{i}", tag="vecP", bufs=2)
                nc.tensor.matmul(ocp[:], osb[:], ones_c[0:1, :], start=True, stop=True)
                oc = apool.tile([128, 1], BF16, name=f"oc_{i}", tag="oc")
                nc.vector.tensor_copy(oc[:], ocp[:])

                # o_proj partials -> [128, G] col form
                wo_t = wpool.tile([128, G, 128], BF16, name=f"wo_{i}", tag="wo")
                nc.sync.dma_start(wo_t[:], wo_in.ap()[:, i])
                pacc = pp.tile([128, G], F32, name=f"pacc_{i}", tag="colP")
                for m in range(G):
                    nc.tensor.matmul(
                        pacc[:, m : m + 1], wo_t[:, m, :], oc[:], start=True, stop=True
                    )
                psb = apool.tile([128, G], F32, name=f"psb_{i}", tag="psb")
                nc.vector.tensor_copy(psb[:], pacc[:])

                # AllReduce #1
                ar1i = dpool.tile([128, G], F32, name=f"ar1i_{i}", bufs=2, tag="ar1i")
                ar1o = dpool.tile(
                    [128, G], F32, addr_space="Shared", name=f"ar1o_{i}",
                    bufs=2, tag="ar1o",
                )
                nc.gpsimd.dma_start(ar1i[:], psb[:])
                nc.gpsimd.collective_compute(
                    "AllReduce", AL.add, replica_groups=[list(range(NC))],
                    ins=[ar1i.opt()], outs=[ar1o.opt()],
                )
                xat = apool.tile([128, G], F32, name=f"xat_{i}", tag="xat")
                nc.gpsimd.dma_start(xat[:], ar1o[:])
                x2 = apool.tile([128, G], F32, name=f"x2_{i}", tag="x", bufs=3)
                nc.vector.tensor_tensor(x2[:], x[:], xat[:], AL.add)
                x = x2

                # ==== MLP ====
                s2, _ = rms_scale(x, f"l{i}m")
                h2 = apool.tile([128, G], F32, name=f"h2_{i}", tag="ht")
                nc.vector.tensor_tensor(h2[:], x[:], lnw[:, :, 2 * i + 1], AL.mult)

                wg_t = wpool.tile([128, G, 512], BF16, name=f"wg_{i}", tag="wg")
                nc.sync.dma_start(wg_t[:], wg_in.ap()[:, i])
                wu_t = wpool.tile([128, G, 512], BF16, name=f"wu_{i}", tag="wu")
                nc.sync.dma_start(wu_t[:], wu_in.ap()[:, i])
                h2b = apool.tile([128, G], BF16, name=f"h2b_{i}", tag="h2b")
                nc.vector.tensor_copy(h2b[:], h2[:])
                warm(ACTF.Silu)
                gate = pp.tile([1, 512], F32, name=f"gate_{i}", tag="rowP", bufs=2)
                up = pp.tile([1, 512], F32, name=f"up_{i}", tag="rowP", bufs=2)
                for g in range(G):
                    nc.tensor.matmul(
                        gate[:], h2b[:, g : g + 1], wg_t[:, g, :],
                        start=(g == 0), stop=(g == G - 1),
                    )
                for g in range(G):
                    nc.tensor.matmul(
                        up[:], h2b[:, g : g + 1], wu_t[:, g, :],
                        start=(g == 0), stop=(g == G - 1),
                    )
                # act = silu(s2*gate) * (s2*up)
                gs = apool.tile([1, 512], F32, name=f"gs_{i}", tag="gs")
                nc.scalar.activation(gs[:], gate[:], ACTF.Silu, scale=s2[:])
                warm(ACTF.Sqrt)
                arow = apool.tile([1, 512], F32, name=f"arow_{i}", tag="arow")
                nc.vector.scalar_tensor_tensor(
                    arow[:], up[:], s2[:], gs[:], AL.mult, AL.mult
                )
                # transpose act to columns [128, FG]
                atp = pp.tile([128, FG], F32, name=f"atp_{i}", tag="vecP", bufs=2)
                for g in range(FG):
                    nc.tensor.matmul(
                        atp[:, g : g + 1], arow[:, g * 128 : (g + 1) * 128],
                        ones_c[0:1, :], start=True, stop=True,
                    )
                ac = apool.tile([128, FG], BF16, name=f"ac_{i}", tag="ac")
                nc.vector.tensor_copy(ac[:], atp[:])

                # down partials -> [128, G] col form
                wd_t = wpool.tile([128, FG, G, 128], BF16, name=f"wd_{i}", tag="wd")
                nc.sync.dma_start(wd_t[:], wd_in.ap()[:, i])
                dacc = pp.tile([128, G], F32, name=f"dacc_{i}", tag="colP")
                for m in range(G):
                    for g in range(FG):
                        nc.tensor.matmul(
                            dacc[:, m : m + 1], wd_t[:, g, m, :], ac[:, g : g + 1],
                            start=(g == 0), stop=(g == FG - 1),
                        )
                dsb = apool.tile([128, G], F32, name=f"dsb_{i}", tag="psb")
                nc.vector.tensor_copy(dsb[:], dacc[:])

                # AllReduce #2
                ar2i = dpool.tile([128, G], F32, name=f"ar2i_{i}", bufs=2, tag="ar2i")
                ar2o = dpool.tile(
                    [128, G], F32, addr_space="Shared", name=f"ar2o_{i}",
                    bufs=2, tag="ar2o",
                )
                nc.gpsimd.dma_start(ar2i[:], dsb[:])
                nc.gpsimd.collective_compute(
                    "AllReduce", AL.add, replica_groups=[list(range(NC))],
                    ins=[ar2i.opt()], outs=[ar2o.opt()],
                )
                xmt = apool.tile([128, G], F32, name=f"xmt_{i}", tag="xat")
                nc.gpsimd.dma_start(xmt[:], ar2o[:])
                x3 = apool.tile([128, G], F32, name=f"x3_{i}", tag="x", bufs=3)
                nc.vector.tensor_tensor(x3[:], x[:], xmt[:], AL.add)
                x = x3

            # ==== final norm + lm heads ====
            _, sfbc = rms_scale(x, "fin")
            hf = apool.tile([128, G], F32, name="hf", tag="hf2")
            nc.vector.tensor_tensor(hf[:], x[:], lnw[:, :, 2 * L], AL.mult)
            hfb = apool.tile([128, G], BF16, name="hfb", tag="hfb")
            nc.vector.tensor_scalar(hfb[:], hf[:], sfbc[:], None, AL.mult)

            for r in range(LMR):
                lm_t = wpool.tile([128, G, LMN], BF16, name=f"lm_{r}", tag="lmr", bufs=8)
                nc.sync.dma_start(lm_t[:], lm_in.ap()[:, :, r, :])
                lg = pp.tile([1, 512], F32, name=f"lg_{r}", tag="rowP", bufs=2)
                for g in range(G):
                    nc.tensor.matmul(
                        lg[:, 0:LMN], hfb[:, g : g + 1], lm_t[:, g, :],
                        start=(g == 0), stop=(g == G - 1),
                    )
                lgs = apool.tile([1, LMN], F32, name=f"lgs_{r}", tag="lgs", bufs=2)
                nc.vector.tensor_copy(lgs[:], lg[:, 0:LMN])
                nc.sync.dma_start(lgt_out.ap()[r], lgs[:])

    nc.compile()
    return nc


def _get_compiled():
    global _COMPILED
    if _COMPILED is None:
        _COMPILED = _build()
    return _COMPILED


def _prep_inputs(input_embed, kv_cache, position, ln1_w, Wq, Wk, Wv, Wo, ln2_w,
                 Wg, Wu, Wd, norm_w, lm_head_w):
    """Build the 8 per-core input dicts (host-side shard + transpose)."""
    f32 = np.float32
    C = np.ascontiguousarray

    pos = float(np.asarray(position).reshape(-1)[0])
    inv_freq = 1.0 / (BASE ** (np.arange(0, D, 2, dtype=np.float64) / D))
    ang = np.concatenate([pos * inv_freq, pos * inv_freq]).astype(np.float64)
    cos = np.cos(ang).astype(f32)
    sin = np.sin(ang).astype(f32)
    sinm = np.concatenate([-sin[0:64], sin[64:128]])
    cskr = np.concatenate([cos, sinm]).reshape(1, 256).astype(f32)
    csqr = (cskr / np.sqrt(np.float32(D))).astype(f32)

    x0 = C(np.asarray(input_embed, f32).reshape(G, 128).T)

    lnstack = np.empty((2 * L + 1, HID), f32)
    for i in range(L):
        lnstack[2 * i] = ln1_w[i]
        lnstack[2 * i + 1] = ln2_w[i]
    lnstack[2 * L] = norm_w
    lnw = C(lnstack.T.reshape(G, 128, 2 * L + 1).transpose(1, 0, 2))

    in_maps = []
    for c in range(NC):
        qkv_c = np.concatenate(
            [
                Wq[:, c * 128 : (c + 1) * 128, :],
                Wk[:, c * 128 : (c + 1) * 128, :],
                Wv[:, c * 128 : (c + 1) * 128, :],
            ],
            axis=1,
        )  # [L, 384, HID]
        wqkv = C(
            qkv_c.transpose(0, 2, 1).reshape(L, G, 128, 384).transpose(2, 0, 1, 3)
        ).astype(ml_dtypes.bfloat16)

        wo = C(
            Wo[:, :, c * 128 : (c + 1) * 128]
            .transpose(2, 0, 1)
            .reshape(128, L, G, 128)
        ).astype(ml_dtypes.bfloat16)

        wg = C(
            Wg[:, c * 512 : (c + 1) * 512, :]
            .transpose(0, 2, 1)
            .reshape(L, G, 128, 512)
            .transpose(2, 0, 1, 3)
        ).astype(ml_dtypes.bfloat16)
        wu = C(
            Wu[:, c * 512 : (c + 1) * 512, :]
            .transpose(0, 2, 1)
            .reshape(L, G, 128, 512)
            .transpose(2, 0, 1, 3)
        ).astype(ml_dtypes.bfloat16)

        wd = C(
            Wd[:, :, c * 512 : (c + 1) * 512]
            .transpose(2, 0, 1)
            .reshape(FG, 128, L, G, 128)
            .transpose(1, 2, 0, 3, 4)
        ).astype(ml_dtypes.bfloat16)

        kt = C(np.asarray(kv_cache)[0::2, 0, c].transpose(0, 2, 1)).astype(ml_dtypes.bfloat16)
        vc = C(np.asarray(kv_cache)[1::2, 0, c].reshape(L, SB, 128, 128)).astype(ml_dtypes.bfloat16)

        lmw = C(
            lm_head_w[:, c * 256 : (c + 1) * 256, :]
            .transpose(2, 0, 1)
            .reshape(HID, NCB * 256)
            .reshape(G, 128, LMR, LMN)
            .transpose(1, 0, 2, 3)
        ).astype(ml_dtypes.bfloat16)

        in_maps.append(
            {
                "x0": x0,
                "csq": csqr,
                "csk": cskr,
                "lnw": lnw,
                "wqkv": wqkv,
                "wo": wo,
                "wg": wg,
                "wu": wu,
                "wd": wd,
                "kt": kt,
                "vc": vc,
                "lmw": lmw,
            }
        )
    return in_maps


def kernel(**inputs):
    from concourse.bass_utils import run_bass_kernel_spmd

    nc = _get_compiled()
    in_maps = _prep_inputs(**{k: np.asarray(v) for k, v in inputs.items()})
    res = run_bass_kernel_spmd(nc, in_maps, core_ids=list(range(NC)))

    kv_cache = np.asarray(inputs["kv_cache"], np.float32)

    all_logits = np.empty((NCB, 1, V), np.float32)
    for c in range(NC):
        vals = res.results[c]["lgt"].reshape(NCB, 256)
        all_logits[:, 0, c * 256 : (c + 1) * 256] = vals

    new_cache = np.empty((2 * L, 1, KVH, S + 1, D), np.float32)
    new_cache[:, :, :, :S, :] = kv_cache
    for c in range(NC):
        kvn = res.results[c]["kvn"]  # [L, 2, 128]
        for i in range(L):
            new_cache[2 * i, 0, c, S, :] = kvn[i, 0]
            new_cache[2 * i + 1, 0, c, S, :] = kvn[i, 1]

    return all_logits, new_cache


# revision 11
# speedup vs baseline: 1.0439x; 1.0439x over previous
"""Trainium2 Bass kernel for nn_CPDecode: 5-layer single-token decode + 15 LM heads.

Sharding (8 cores, tensor parallel):
  - attention: 1 head per core (H=KVH=8); Wq/Wk/Wv rows, Wo cols, kv_cache head dim
  - MLP: FF/8 = 512 rows of Wg/Wu per core, 512 cols of Wd
  - lm_heads: V/8 = 256 rows of each of the 15 heads per core
  - AllReduce (ncfw) after o_proj and down_proj partials; logits/new-cache
    assembled on host.

All weights are pre-transposed on host so every PE contraction has K on the
partition axis. 1024-vectors live in SBUF as [128, 8] "col-block" tiles:
tile[p, g] = v[g*128 + p].
"""

import sys
import os

sys.path.insert(0, "/opt/trn_rl_repo")

import numpy as np
import ml_dtypes

L, H, KVH, D, HID, FF, V, NCB, S = 5, 8, 8, 128, 1024, 4096, 2048, 15, 4096
BASE, EPS = 10000.0, 1e-6
NC = 8
G = HID // 128  # 8 col-blocks of the residual stream
FG = 512 // 128  # 4 col-blocks of the per-core FF shard
SB = S // 128  # 32 score blocks
LMR = 8  # lm ranges
LMN = 3840 // LMR  # 480 outputs per range

_COMPILED = None


def _build():
    from concourse import bass, bacc, tile, mybir

    F32 = mybir.dt.float32
    BF16 = mybir.dt.bfloat16
    AL = mybir.AluOpType
    ACTF = mybir.ActivationFunctionType

    nc = bacc.Bacc("TRN2", target_bir_lowering=False, debug=False, num_devices=NC)

    # ---- I/O ----
    x0_in = nc.dram_tensor("x0", [128, G], F32, kind="ExternalInput")
    csq_in = nc.dram_tensor("csq", [1, 256], F32, kind="ExternalInput")
    csk_in = nc.dram_tensor("csk", [1, 256], F32, kind="ExternalInput")
    lnw_in = nc.dram_tensor("lnw", [128, G, 2 * L + 1], F32, kind="ExternalInput")
    wqkv_in = nc.dram_tensor("wqkv", [128, L, G, 384], BF16, kind="ExternalInput")
    wo_in = nc.dram_tensor("wo", [128, L, G, 128], BF16, kind="ExternalInput")
    wg_in = nc.dram_tensor("wg", [128, L, G, 512], BF16, kind="ExternalInput")
    wu_in = nc.dram_tensor("wu", [128, L, G, 512], BF16, kind="ExternalInput")
    wd_in = nc.dram_tensor("wd", [128, L, FG, G, 128], BF16, kind="ExternalInput")
    kt_in = nc.dram_tensor("kt", [L, 128, S], BF16, kind="ExternalInput")
    vc_in = nc.dram_tensor("vc", [L, SB, 128, 128], BF16, kind="ExternalInput")
    lm_in = nc.dram_tensor("lmw", [128, G, LMR, LMN], BF16, kind="ExternalInput")

    lgt_out = nc.dram_tensor("lgt", [LMR, LMN], F32, kind="ExternalOutput")
    kvn_out = nc.dram_tensor("kvn", [L, 2, 128], F32, kind="ExternalOutput")

    with tile.TileContext(nc) as tc:
        with (
            tc.tile_pool(name="const", bufs=1) as cpool,
            tc.tile_pool(name="wb", bufs=2) as wpool,
            tc.tile_pool(name="act", bufs=2) as apool,
            tc.tile_pool(name="ps", bufs=1, space="PSUM") as pp,
            tc.tile_pool(name="dram", bufs=1, space="DRAM") as dpool,
        ):
            # ---- constants ----
            csq = cpool.tile([1, 256], F32)
            nc.sync.dma_start(csq[:], csq_in.ap())
            csk = cpool.tile([1, 256], F32)
            nc.sync.dma_start(csk[:], csk_in.ap())
            lnw = cpool.tile([128, G, 2 * L + 1], F32)
            nc.sync.dma_start(lnw[:], lnw_in.ap())
            ones_c = cpool.tile([128, 1], F32)
            nc.vector.memset(ones_c[:], 1.0)
            ones_r = cpool.tile([1, 128], F32)
            nc.vector.memset(ones_r[:], 1.0)
            eps_t = cpool.tile([1, 1], F32)
            nc.vector.memset(eps_t[:], EPS)

            x = apool.tile([128, G], F32, name="x_init", tag="x", bufs=3)
            nc.sync.dma_start(x[:], x0_in.ap())

            # ACT table pre-warm scratch
            junk = cpool.tile([1, 1], F32)
            nc.vector.memset(junk[:], 1.0)
            junko = cpool.tile([1, 4], F32)

            def warm(func):
                nc.scalar.activation(junko[:, 0:1], junk[:], func)

            def rms_scale(xt, tag):
                """rsqrt(mean(x^2)+eps) -> ([1,1] sbuf, [128,1] sbuf bcast)"""
                sq = apool.tile([128, G], F32, name=f"sq_{tag}", tag="sq")
                nc.vector.tensor_tensor(sq[:], xt[:], xt[:], AL.mult)
                rs = apool.tile([128, 1], F32, name=f"rs_{tag}", tag="rs")
                nc.vector.tensor_reduce(rs[:], sq[:], mybir.AxisListType.X, AL.add)
                ssq = pp.tile([1, 1], F32, name=f"ssq_{tag}", tag="scalP")
                nc.tensor.matmul(ssq[:], rs[:], ones_c[:], start=True, stop=True)
                sr = apool.tile([1, 1], F32, name=f"sr_{tag}", tag="sr", bufs=2)
                nc.scalar.activation(sr[:], ssq[:], ACTF.Sqrt, bias=eps_t[:], scale=1.0 / HID)
                s = apool.tile([1, 1], F32, name=f"s_{tag}", tag="s", bufs=3)
                nc.vector.reciprocal(s[:], sr[:])
                sbcp = pp.tile([128, 1], F32, name=f"sbcp_{tag}", tag="vecP", bufs=2)
                nc.tensor.matmul(sbcp[:], ones_r[:], s[:], start=True, stop=True)
                sbc = apool.tile([128, 1], F32, name=f"sbc_{tag}", tag="sbc", bufs=2)
                nc.vector.tensor_copy(sbc[:], sbcp[:])
                return s, sbc

            def rope_row(dst, src_ap, cs):
                """dst[1,128] = src*cos + rot_half(src)*sin (row form)."""
                qc = apool.tile([1, 128], F32, name=f"{dst.name}_qc", tag="ropeA")
                nc.vector.tensor_tensor(qc[:], src_ap, cs[:, 0:128], AL.mult)
                t = apool.tile([1, 128], F32, name=f"{dst.name}_t", tag="ropeB")
                nc.vector.tensor_tensor(t[:, 0:64], src_ap[:, 64:128], cs[:, 128:192], AL.mult)
                nc.vector.tensor_tensor(t[:, 64:128], src_ap[:, 0:64], cs[:, 192:256], AL.mult)
                nc.vector.tensor_tensor(dst[:], qc[:], t[:], AL.add)

            for i in range(L):
                # ==== attention ====
                s1, s1bc = rms_scale(x, f"l{i}a")
                ht = apool.tile([128, G], BF16, name=f"ht_{i}", tag="ht")
                nc.vector.tensor_tensor(ht[:], x[:], lnw[:, :, 2 * i], AL.mult)
                warm(ACTF.Exp)

                wqkv_t = wpool.tile([128, G, 384], BF16, name=f"wqkv_{i}", tag="wqkv")
                nc.sync.dma_start(wqkv_t[:], wqkv_in.ap()[:, i])
                qkv = pp.tile([1, 384], F32, name=f"qkv_{i}", tag="qkvrow")
                for g in range(G):
                    nc.tensor.matmul(
                        qkv[:], ht[:, g : g + 1], wqkv_t[:, g, :],
                        start=(g == 0), stop=(g == G - 1),
                    )

                # rope (row form, unscaled by s1)
                qr = apool.tile([1, 128], F32, name=f"qr_{i}", tag="qr")
                rope_row(qr, qkv[:, 0:128], csq)
                kr = apool.tile([1, 128], F32, name=f"kr_{i}", tag="kr")
                rope_row(kr, qkv[:, 128:256], csk)

                # kv outputs: true k/v = s1 * (unscaled)
                kn = apool.tile([1, 128], F32, name=f"kn_{i}", tag="kn")
                nc.vector.tensor_scalar(kn[:], kr[:], s1[:], None, AL.mult)
                vn = apool.tile([1, 128], F32, name=f"vn_{i}", tag="vn")
                nc.vector.tensor_scalar(vn[:], qkv[:, 256:384], s1[:], None, AL.mult)
                nc.sync.dma_start(kvn_out.ap()[i, 0], kn[:])
                nc.sync.dma_start(kvn_out.ap()[i, 1], vn[:])

                # transpose qr, kr to columns
                qrcp = pp.tile([128, 1], F32, name=f"qrcp_{i}", tag="vecP", bufs=2)
                nc.tensor.matmul(qrcp[:], qr[:], ones_c[0:1, :], start=True, stop=True)
                qrc = apool.tile([128, 1], BF16, name=f"qrc_{i}", tag="qrc")
                nc.vector.tensor_copy(qrc[:], qrcp[:])
                krcp = pp.tile([128, 1], F32, name=f"krcp_{i}", tag="vecP", bufs=2)
                nc.tensor.matmul(krcp[:], kr[:], ones_c[0:1, :], start=True, stop=True)
                krc = apool.tile([128, 1], BF16, name=f"krc_{i}", tag="krc")
                nc.vector.tensor_copy(krc[:], krcp[:])

                # scores: [128, 32] blocks vs cached K^T
                ktile = wpool.tile([128, S], BF16, name=f"kt_{i}", tag="kt", bufs=1)
                nc.sync.dma_start(ktile[:], kt_in.ap()[i])
                sacc = pp.tile([128, SB], F32, name=f"sacc_{i}", tag="sacc")
                for b in range(SB):
                    nc.tensor.matmul(
                        sacc[:, b : b + 1], ktile[:, b * 128 : (b + 1) * 128],
                        qrc[:], start=True, stop=True,
                    )
                snew = pp.tile([1, 1], F32, name=f"snew_{i}", tag="scalP")
                nc.tensor.matmul(snew[:], qrc[:], krc[:], start=True, stop=True)

                # softmax (unnormalized; scale folds s1 back in)
                es = apool.tile([128, SB], BF16, name=f"es_{i}", tag="es")
                rowsum = apool.tile([128, 1], F32, name=f"rsum_{i}", tag="rsum")
                nc.scalar.activation(
                    es[:], sacc[:], ACTF.Exp, scale=s1bc[:], accum_out=rowsum[:]
                )
                s1sq = apool.tile([1, 1], F32, name=f"s1sq_{i}", tag="s1sq")
                nc.vector.tensor_tensor(s1sq[:], s1[:], s1[:], AL.mult)
                enew = apool.tile([1, 1], F32, name=f"enew_{i}", tag="enew")
                nc.scalar.activation(enew[:], snew[:], ACTF.Exp, scale=s1sq[:])
                warm(ACTF.Sqrt)
                dsum = pp.tile([1, 1], F32, name=f"dsum_{i}", tag="scalP")
                nc.tensor.matmul(dsum[:], rowsum[:], ones_c[:], start=True, stop=True)
                dtot = apool.tile([1, 1], F32, name=f"dtot_{i}", tag="dtot")
                nc.vector.tensor_tensor(dtot[:], dsum[:], enew[:], AL.add)
                rr = apool.tile([1, 1], F32, name=f"rr_{i}", tag="rr")
                nc.vector.reciprocal(rr[:], dtot[:])

                # attn @ V -> o row [1, 128]
                vtile = wpool.tile([128, SB, 128], BF16, name=f"v_{i}", tag="vt", bufs=1)
                nc.sync.dma_start(vtile[:], vc_in.ap()[i].rearrange("b s d -> s b d"))
                orow = pp.tile([1, 128], F32, name=f"orow_{i}", tag="qkvrow")
                for b in range(SB):
                    nc.tensor.matmul(
                        orow[:], es[:, b : b + 1], vtile[:, b, :],
                        start=(b == 0), stop=(b == SB - 1),
                    )
                # o = (orow + enew * vn) * rr
                osb = apool.tile([1, 128], F32, name=f"osb_{i}", tag="osb")
                nc.vector.scalar_tensor_tensor(
                    osb[:], vn[:], enew[:], orow[:], AL.mult, AL.add
                )
                nc.vector.tensor_scalar(osb[:], osb[:], rr[:], None, AL.mult)
                # transpose to column
                ocp = pp.tile([128, 1], F32, name=f"ocp_{i}", tag="vecP", bufs=2)
                nc.tensor.matmul(ocp[:], osb[:], ones_c[0:1, :], start=True, stop=True)
                oc = apool.tile([128, 1], BF16, name=f"oc_{i}", tag="oc")
                nc.vector.tensor_copy(oc[:], ocp[:])

                # o_proj partials -> [128, G] col form
                wo_t = wpool.tile([128, G, 128], BF16, name=f"wo_{i}", tag="wo")
                nc.sync.dma_start(wo_t[:], wo_in.ap()[:, i])
                pacc = pp.tile([128, G], F32, name=f"pacc_{i}", tag="colP")
                for m in range(G):
                    nc.tensor.matmul(
                        pacc[:, m : m + 1], wo_t[:, m, :], oc[:], start=True, stop=True
                    )
                psb = apool.tile([128, G], F32, name=f"psb_{i}", tag="psb")
                nc.vector.tensor_copy(psb[:], pacc[:])

                # AllReduce #1
                ar1i = dpool.tile([128, G], F32, name=f"ar1i_{i}", bufs=2, tag="ar1i")
                ar1o = dpool.tile(
                    [128, G], F32, addr_space="Shared", name=f"ar1o_{i}",
                    bufs=2, tag="ar1o",
                )
                nc.scalar.dma_start(ar1i[:], psb[:])
                nc.gpsimd.collective_compute(
                    "AllReduce", AL.add, replica_groups=[list(range(NC))],
                    ins=[ar1i.opt()], outs=[ar1o.opt()],
                )
                xat = apool.tile([128, G], F32, name=f"xat_{i}", tag="xat")
                nc.scalar.dma_start(xat[:], ar1o[:])
                x2 = apool.tile([128, G], F32, name=f"x2_{i}", tag="x", bufs=3)
                nc.vector.tensor_tensor(x2[:], x[:], xat[:], AL.add)
                x = x2

                # ==== MLP ====
                s2, _ = rms_scale(x, f"l{i}m")
                h2 = apool.tile([128, G], F32, name=f"h2_{i}", tag="ht")
                nc.vector.tensor_tensor(h2[:], x[:], lnw[:, :, 2 * i + 1], AL.mult)

                wg_t = wpool.tile([128, G, 512], BF16, name=f"wg_{i}", tag="wg")
                nc.sync.dma_start(wg_t[:], wg_in.ap()[:, i])
                wu_t = wpool.tile([128, G, 512], BF16, name=f"wu_{i}", tag="wu")
                nc.sync.dma_start(wu_t[:], wu_in.ap()[:, i])
                h2b = apool.tile([128, G], BF16, name=f"h2b_{i}", tag="h2b")
                nc.vector.tensor_copy(h2b[:], h2[:])
                warm(ACTF.Silu)
                gate = pp.tile([1, 512], F32, name=f"gate_{i}", tag="rowP", bufs=2)
                up = pp.tile([1, 512], F32, name=f"up_{i}", tag="rowP", bufs=2)
                for g in range(G):
                    nc.tensor.matmul(
                        gate[:], h2b[:, g : g + 1], wg_t[:, g, :],
                        start=(g == 0), stop=(g == G - 1),
                    )
                for g in range(G):
                    nc.tensor.matmul(
                        up[:], h2b[:, g : g + 1], wu_t[:, g, :],
                        start=(g == 0), stop=(g == G - 1),
                    )
                # act = silu(s2*gate) * (s2*up)
                gs = apool.tile([1, 512], F32, name=f"gs_{i}", tag="gs")
                nc.scalar.activation(gs[:], gate[:], ACTF.Silu, scale=s2[:])
                warm(ACTF.Sqrt)
                arow = apool.tile([1, 512], F32, name=f"arow_{i}", tag="arow")
                nc.vector.scalar_tensor_tensor(
                    arow[:], up[:], s2[:], gs[:], AL.mult, AL.mult
                )
                # transpose act to columns [128, FG]
                atp = pp.tile([128, FG], F32, name=f"atp_{i}", tag="vecP", bufs=2)
                for g in range(FG):
                    nc.tensor.matmul(
                        atp[:, g : g + 1], arow[:, g * 128 : (g + 1) * 128],
                        ones_c[0:1, :], start=True, stop=True,
                    )
                ac = apool.tile([128, FG], BF16, name=f"ac_{i}", tag="ac")
                nc.vector.tensor_copy(ac[:], atp[:])

                # down partials -> [128, G] col form
                wd_t = wpool.tile([128, FG, G, 128], BF16, name=f"wd_{i}", tag="wd")
                nc.sync.dma_start(wd_t[:], wd_in.ap()[:, i])
                dacc = pp.tile([128, G], F32, name=f"dacc_{i}", tag="colP")
                for m in range(G):
                    for g in range(FG):
                        nc.tensor.matmul(
                            dacc[:, m : m + 1], wd_t[:, g, m, :], ac[:, g : g + 1],
                            start=(g == 0), stop=(g == FG - 1),
                        )
                dsb = apool.tile([128, G], F32, name=f"dsb_{i}", tag="psb")
                nc.vector.tensor_copy(dsb[:], dacc[:])

                # AllReduce #2
                ar2i = dpool.tile([128, G], F32, name=f"ar2i_{i}", bufs=2, tag="ar2i")
                ar2o = dpool.tile(
                    [128, G], F32, addr_space="Shared", name=f"ar2o_{i}",
                    bufs=2, tag="ar2o",
                )
                nc.scalar.dma_start(ar2i[:], dsb[:])
                nc.gpsimd.collective_compute(
                    "AllReduce", AL.add, replica_groups=[list(range(NC))],
                    ins=[ar2i.opt()], outs=[ar2o.opt()],
                )
                xmt = apool.tile([128, G], F32, name=f"xmt_{i}", tag="xat")
                nc.scalar.dma_start(xmt[:], ar2o[:])
                x3 = apool.tile([128, G], F32, name=f"x3_{i}", tag="x", bufs=3)
                nc.vector.tensor_tensor(x3[:], x[:], xmt[:], AL.add)
                x = x3

            # ==== final norm + lm heads ====
            _, sfbc = rms_scale(x, "fin")
            hf = apool.tile([128, G], F32, name="hf", tag="hf2")
            nc.vector.tensor_tensor(hf[:], x[:], lnw[:, :, 2 * L], AL.mult)
            hfb = apool.tile([128, G], BF16, name="hfb", tag="hfb")
            nc.vector.tensor_scalar(hfb[:], hf[:], sfbc[:], None, AL.mult)

            for r in range(LMR):
                lm_t = wpool.tile([128, G, LMN], BF16, name=f"lm_{r}", tag="lmr", bufs=8)
                nc.sync.dma_start(lm_t[:], lm_in.ap()[:, :, r, :])
                lg = pp.tile([1, 512], F32, name=f"lg_{r}", tag="rowP", bufs=2)
                for g in range(G):
                    nc.tensor.matmul(
                        lg[:, 0:LMN], hfb[:, g : g + 1], lm_t[:, g, :],
                        start=(g == 0), stop=(g == G - 1),
                    )
                lgs = apool.tile([1, LMN], F32, name=f"lgs_{r}", tag="lgs", bufs=2)
                nc.vector.tensor_copy(lgs[:], lg[:, 0:LMN])
                nc.sync.dma_start(lgt_out.ap()[r], lgs[:])

    nc.compile()
    return nc


def _get_compiled():
    global _COMPILED
    if _COMPILED is None:
        _COMPILED = _build()
    return _COMPILED


def _prep_inputs(input_embed, kv_cache, position, ln1_w, Wq, Wk, Wv, Wo, ln2_w,
                 Wg, Wu, Wd, norm_w, lm_head_w):
    """Build the 8 per-core input dicts (host-side shard + transpose)."""
    f32 = np.float32
    C = np.ascontiguousarray

    pos = float(np.asarray(position).reshape(-1)[0])
    inv_freq = 1.0 / (BASE ** (np.arange(0, D, 2, dtype=np.float64) / D))
    ang = np.concatenate([pos * inv_freq, pos * inv_freq]).astype(np.float64)
    cos = np.cos(ang).astype(f32)
    sin = np.sin(ang).astype(f32)
    sinm = np.concatenate([-sin[0:64], sin[64:128]])
    cskr = np.concatenate([cos, sinm]).reshape(1, 256).astype(f32)
    csqr = (cskr / np.sqrt(np.float32(D))).astype(f32)

    x0 = C(np.asarray(input_embed, f32).reshape(G, 128).T)

    lnstack = np.empty((2 * L + 1, HID), f32)
    for i in range(L):
        lnstack[2 * i] = ln1_w[i]
        lnstack[2 * i + 1] = ln2_w[i]
    lnstack[2 * L] = norm_w
    lnw = C(lnstack.T.reshape(G, 128, 2 * L + 1).transpose(1, 0, 2))

    in_maps = []
    for c in range(NC):
        qkv_c = np.concatenate(
            [
                Wq[:, c * 128 : (c + 1) * 128, :],
                Wk[:, c * 128 : (c + 1) * 128, :],
                Wv[:, c * 128 : (c + 1) * 128, :],
            ],
            axis=1,
        )  # [L, 384, HID]
        wqkv = C(
            qkv_c.transpose(0, 2, 1).reshape(L, G, 128, 384).transpose(2, 0, 1, 3)
        ).astype(ml_dtypes.bfloat16)

        wo = C(
            Wo[:, :, c * 128 : (c + 1) * 128]
            .transpose(2, 0, 1)
            .reshape(128, L, G, 128)
        ).astype(ml_dtypes.bfloat16)

        wg = C(
            Wg[:, c * 512 : (c + 1) * 512, :]
            .transpose(0, 2, 1)
            .reshape(L, G, 128, 512)
            .transpose(2, 0, 1, 3)
        ).astype(ml_dtypes.bfloat16)
        wu = C(
            Wu[:, c * 512 : (c + 1) * 512, :]
            .transpose(0, 2, 1)
            .reshape(L, G, 128, 512)
            .transpose(2, 0, 1, 3)
        ).astype(ml_dtypes.bfloat16)

        wd = C(
            Wd[:, :, c * 512 : (c + 1) * 512]
            .transpose(2, 0, 1)
            .reshape(FG, 128, L, G, 128)
            .transpose(1, 2, 0, 3, 4)
        ).astype(ml_dtypes.bfloat16)

        kt = C(np.asarray(kv_cache)[0::2, 0, c].transpose(0, 2, 1)).astype(ml_dtypes.bfloat16)
        vc = C(np.asarray(kv_cache)[1::2, 0, c].reshape(L, SB, 128, 128)).astype(ml_dtypes.bfloat16)

        lmw = C(
            lm_head_w[:, c * 256 : (c + 1) * 256, :]
            .transpose(2, 0, 1)
            .reshape(HID, NCB * 256)
            .reshape(G, 128, LMR, LMN)
            .transpose(1, 0, 2, 3)
        ).astype(ml_dtypes.bfloat16)

        in_maps.append(
            {
                "x0": x0,
                "csq": csqr,
                "csk": cskr,
                "lnw": lnw,
                "wqkv": wqkv,
                "wo": wo,
                "wg": wg,
                "wu": wu,
                "wd": wd,
                "kt": kt,
                "vc": vc,
                "lmw": lmw,
            }
        )
    return in_maps


def kernel(**inputs):
    from concourse.bass_utils import run_bass_kernel_spmd

    nc = _get_compiled()
    in_maps = _prep_inputs(**{k: np.asarray(v) for k, v in inputs.items()})
    res = run_bass_kernel_spmd(nc, in_maps, core_ids=list(range(NC)))

    kv_cache = np.asarray(inputs["kv_cache"], np.float32)

    all_logits = np.empty((NCB, 1, V), np.float32)
    for c in range(NC):
        vals = res.results[c]["lgt"].reshape(NCB, 256)
        all_logits[:, 0, c * 256 : (c + 1) * 256] = vals

    new_cache = np.empty((2 * L, 1, KVH, S + 1, D), np.float32)
    new_cache[:, :, :, :S, :] = kv_cache
    for c in range(NC):
        kvn = res.results[c]["kvn"]  # [L, 2, 128]
        for i in range(L):
            new_cache[2 * i, 0, c, S, :] = kvn[i, 0]
            new_cache[2 * i + 1, 0, c, S, :] = kvn[i, 1]

    return all_logits, new_cache
